# revision 41
# baseline (speedup 1.0000x reference)
"""Trainium2 Bass kernel for nn_ImitationHead (dense_mlp).

Computation (per batch row b of 256):
  h  = mean(z[b], spatial)                # [512] <- z [512,16,16]
  h  = relu-MLP chain 512->512->256->128->64
  goal = [goal_point[b,0,3], goal_point[b,1,3], goal_point_speed[b]]
  GRU (hidden 64, input [x(3); goal(3)]) unrolled 8 steps, each step
  followed by an output MLP 64->4(relu)->4->3 producing dx; x += dx.
  Output: the 8 x values -> [256, 8, 3].

Sharding: pure data parallel, batch 256 -> 8 cores x 32.

Key layout/perf choices:
  - z and all weights travel as float16: halves the HBM traffic that
    dominates the kernel (8 MiB z + ~0.9 MiB weights per core).  fp16
    keeps 10 mantissa bits so the 2e-2 output tolerance is safe
    (measured rel err ~7e-4).
  - on-chip layout fully "transposed" (features on partitions, batch on
    the free axis); z shard viewed as [16384, 256] f16, 7 DMAs of
    [128p, 4, 4, 256] (1 MiB) + 2 half-size DMAs, with 2 KiB contiguous
    DRAM runs.  The channel permutation (chunk j, partition p <->
    channel 4p+j) is undone by permuting W1's rows on the host.
  - spatial sums: TensorReduce runs at 1x on DVE but fp16 TensorTensor
    gets the 2x perf mode, so most chunks reduce via a binary tree of
    fp16 adds on DVE (4 halving stages + one 16-wide reduce); the
    remainder go to ACT as Copy+accum_out chunks.  Both engines stay
    just under the 2.9us-per-DMA stream rate, and the last z DMA is
    split in half so the final tree only trails the stream slightly.
  - weights are loaded AFTER the z stream (they are not needed until
    then), ordered by first use: w1, biases, w2, w34, GRU pack.
  - join MLP in fp16 (1 PE cycle/row): biases fold in as K=1 matmuls
    (bias row x ones row) accumulating into the same psum bank as the
    layer, all m-groups of a layer share ONE psum tile, and a single
    DVE max(.,0) per layer emits the fp16 activations.
  - GRU: persistent PSUM accumulators; hh' = hh - d with
    d = (1-z)*(hh-n); the x-recurrence folds through the output MLP
    (wixo = W23 @ W_ihx.T applied to relu(pd1)).  Per step: ACT does
    sigmoid(r), sigmoid(-prz_z) (= 1-z, off-chain) and tanh; the whole
    elementwise chain runs on DVE in fp16 SBUF tiles (2x perf mode,
    same-engine ordering instead of cross-engine semaphores).  The
    i_n / h_n psum accumulators are shadowed into fp16 SBUF copies
    off-chain right after their matmul updates, so no chain op ever
    waits on a PE semaphore or pays a PSUM access except the forced
    pd1 relu.  Per-step incremental matmuls are fp16.  Biases fold in
    as an extra all-ones input row at init; the 4->4 and 4->3 output
    layers fold into one 4->3 matrix on the host; the spatial-mean
    1/256 folds into W1.
"""

import numpy as np
from contextlib import ExitStack

N_CORES = 8
B = 256
B_SH = B // N_CORES       # 32 batch rows per core
C = 512                   # channels
S = 256                   # spatial 16*16
HID = 64
T = 8                     # pred_len
ROWS = B_SH * C           # 16384 z rows per core
N_DMA = 8                 # z DMAs per core (1 MiB f16 each)
H_PER = 4                 # batch blocks per z DMA
J = 4                     # 256-chunks per partition per batch block

# f16 constant pack [65, 939]
P16_COLS = 939

_CACHE: dict = {}


def _build_program():
    import concourse.bacc as bacc
    import concourse.tile as tile
    from concourse import mybir

    f32 = mybir.dt.float32
    f16 = mybir.dt.float16
    AF = mybir.ActivationFunctionType
    AX = mybir.AxisListType
    ALU = mybir.AluOpType

    nc = bacc.Bacc("TRN2", target_bir_lowering=False, debug=False)

    z = nc.dram_tensor("z", [ROWS, S], f16, kind="ExternalInput")
    w1d = nc.dram_tensor("w1", [512, 512], f16, kind="ExternalInput")
    w2d = nc.dram_tensor("w2", [512, 256], f16, kind="ExternalInput")
    w34d = nc.dram_tensor("w34", [128, 320], f16, kind="ExternalInput")
    wsm_d = nc.dram_tensor("wsm", [1, 1024], f16, kind="ExternalInput")
    wp16_d = nc.dram_tensor("wp16", [65, P16_COLS], f16, kind="ExternalInput")
    out_d = nc.dram_tensor("out", [3 * T, B_SH], f32, kind="ExternalOutput")

    with tile.TileContext(nc) as tc, ExitStack() as ctx, \
            nc.allow_low_precision(reason="fp16 pipeline; output tol 2e-2"):
        consts = ctx.enter_context(tc.tile_pool(name="consts", bufs=1))
        zpool = ctx.enter_context(tc.tile_pool(name="zpool", bufs=3))
        hpool = ctx.enter_context(tc.tile_pool(name="hpool", bufs=1))
        work = ctx.enter_context(tc.tile_pool(name="work", bufs=2))
        gwork = ctx.enter_context(tc.tile_pool(name="gwork", bufs=8))
        xpool = ctx.enter_context(tc.tile_pool(name="xpool", bufs=2))
        psum_mlp = ctx.enter_context(
            tc.tile_pool(name="psum_mlp", bufs=2, space="PSUM"))
        psum_gru = ctx.enter_context(
            tc.tile_pool(name="psum_gru", bufs=1, space="PSUM"))

        wsm = consts.tile([1, 1024], f16)
        wp16 = consts.tile([65, P16_COLS], f16)

        whhn = wp16[0:64, 0:192]
        wixo = wp16[0:33, 192:384]
        ow1n = wp16[0:64, 384:388]
        ow23 = wp16[0:33, 388:391]
        id64 = wp16[0:64, 391:455]
        id64n = wp16[0:64, 455:519]
        whh = wp16[0:65, 519:711]
        wgo = wp16[0:4, 711:903]
        gl = wp16[0:4, 903:903 + B_SH]
        ow1 = wp16[0:65, 935:939]

        # ACT table warmup: sigmoid/tanh tables resident before the tail.
        warm = consts.tile([1, 1], f32)
        nc.vector.memset(warm, 0.0)
        nc.scalar.activation(warm, warm, AF.Sigmoid)
        nc.scalar.activation(warm, warm, AF.Tanh)
        # PE p-state warmup: one early matmul starts the clock-ramp window
        # so the MLP's matmuls run at full speed (prz is reset by its
        # start=True init matmul later).
        warm16 = consts.tile([1, 1], f16)
        nc.vector.memset(warm16, 0.0)

        # hhg rows 0:64 = GRU hidden state (in-place across steps), row 64 = 1.
        hhg = hpool.tile([65, B_SH], f16)
        nc.vector.memset(hhg[64:65, :], 1.0)
        # d1g: relu(pd1) with ones row at partition 32; rows 4:32 stay zero
        # so the K=33 matmuls see only d1 + bias.
        d1g = hpool.tile([33, B_SH], f16)
        nc.vector.memset(d1g[0:33, :], 0.0)
        nc.vector.memset(d1g[32:33, :], 1.0)

        kw = dict(skip_group_check=True)
        prz = psum_gru.tile([128, B_SH], f32, tag="prz")   # r/z pre-act
        nc.tensor.matmul(prz[0:1, 0:1], warm16, warm16, start=True, stop=True,
                         **kw)
        pin = psum_gru.tile([64, B_SH], f32, tag="pin")    # i_n pre-act
        phn = psum_gru.tile([64, B_SH], f32, tag="phn")    # h_n pre-act
        pd1 = psum_gru.tile([4, B_SH], f32, tag="pd1")     # oW1@hh+ob1

        # --- z stream: 8 x 1MiB f16 DMAs.  TensorReduce runs at 1x on
        # DVE but TensorTensor fp16 gets the 2x perf mode, so the spatial
        # sum is mostly a binary tree of fp16 adds on DVE (13 of 16 rows
        # per DMA); the remaining 3 chunks go to ACT (Copy+accum_out).
        # Both engines stay just under the 2.9us DMA time, and the last
        # DMA is split in half so the final tree only trails the stream
        # slightly.
        # Row d*2048 + h*512 + 4p + j -> batch b = 4d+h, channel 4p+j.
        hTc = hpool.tile([128, B_SH, J], f16)
        junk_a = hpool.tile([128, S], f16)
        z_r = z[:].rearrange("(d h p j) s -> d p h j s", h=H_PER, p=128, j=J)

        def tree_reduce(src_ap, n_rows, out_ap):
            # src_ap [128, n_rows, S] f16 -> out_ap [128, n_rows, 1]:
            # 4 halving TT stages then one 1x multi-axis reduce of 16.
            scA = work.tile([128, n_rows, S // 2], f16, tag=f"trA{n_rows}")
            scB = work.tile([128, n_rows, S // 4], f16, tag=f"trB{n_rows}")
            n = S // 2
            nc.vector.tensor_add(scA[:, :, 0:n], src_ap[:, :, 0:n],
                                 src_ap[:, :, n:2 * n])
            cur, oth = scA, scB
            while n > 16:
                h_n = n // 2
                nc.vector.tensor_add(oth[:, :, 0:h_n], cur[:, :, 0:h_n],
                                     cur[:, :, h_n:n])
                cur, oth = oth, cur
                n = h_n
            nc.vector.tensor_reduce(out=out_ap, in_=cur[:, :, 0:16],
                                    axis=AX.X, op=ALU.add)

        def act_chunks(zt, h, b, js):
            for j in js:
                nc.scalar.activation(
                    out=junk_a, in_=zt[:, h, j, :], func=AF.Copy,
                    accum_out=hTc[:, b, j:j + 1])

        for d in range(N_DMA - 1):
            zt = zpool.tile([128, H_PER, J, S], f16, tag="zt")
            nc.sync.dma_start(out=zt, in_=z_r[d])
            b = H_PER * d
            tree_reduce(
                zt[:].rearrange("p h j s -> p (h j) s")[:, 0:13, :], 13,
                hTc[:, b:b + 4, :].rearrange("p b j -> p (b j) ()")[:, 0:13, :])
            act_chunks(zt, 3, b + 3, range(1, 4))
            # keep the PE clock-ramp window alive through the stream
            nc.tensor.matmul(prz[0:1, 0:1], zt[0:1, 0, 0, 0:1], warm16,
                             start=True, stop=True, **kw)
        d = N_DMA - 1
        for half in range(2):
            zh = zpool.tile([128, 2, J, S], f16, tag="zh")
            nc.sync.dma_start(out=zh, in_=z_r[d][:, 2 * half:2 * half + 2])
            b = H_PER * d + 2 * half
            # DVE: row 0 fully + half of row 1; ACT: the other half
            tree_reduce(
                zh[:].rearrange("p h j s -> p (h j) s")[:, 0:6, :], 6,
                hTc[:, b:b + 2, :].rearrange("p b j -> p (b j) ()")[:, 0:6, :])
            act_chunks(zh, 1, b + 1, range(2, 4))
        # --- weights queued after the z stream, in order of first use:
        # w1, biases (L1 relu), w2, w34, then the GRU packs.
        w1 = consts.tile([128, 4, 512], f16)
        jw1_r = w1d[:].rearrange("(k p) m -> k p m", p=128)
        for k in range(0, 4, 2):
            nc.sync.dma_start(out=w1[:, k:k + 2, :], in_=jw1_r[k:k + 2])
        nc.sync.dma_start(out=wsm, in_=wsm_d[:])
        w2 = consts.tile([128, 4, 256], f16)
        nc.sync.dma_start(out=w2, in_=w2d[:].rearrange("(k p) m -> p k m", p=128))
        w34 = consts.tile([128, 320], f16)
        nc.sync.dma_start(out=w34, in_=w34d[:])
        nc.sync.dma_start(out=wp16, in_=wp16_d[:])
        # GRU goal-path init matmuls
        nc.tensor.matmul(prz, wgo[:, 0:128], gl, start=True, stop=False, **kw)
        nc.tensor.matmul(pin, wgo[:, 128:192], gl, start=True, stop=False, **kw)

        # --- join MLP (transposed): hN_T = relu(W @ h_T + b) ---
        # bias+relu fused on Pool: (psum + bias) max 0 -> f16
        # --- join MLP: bias folded in as K=1 matmuls (bias x ones row),
        # all m-groups of a layer accumulate into ONE psum bank, and a
        # single DVE max(.,0) per layer writes the fp16 activations.
        jb1 = wsm[0:1, 0:512]
        jb2 = wsm[0:1, 512:768]
        jb3 = wsm[0:1, 768:896]
        jb4 = wsm[0:1, 896:960]
        one_r = wsm[0:1, 960:992]

        h1 = hpool.tile([128, 4, B_SH], f16)
        pt4 = psum_mlp.tile([128, 4, B_SH], f32, tag="mlp")
        for m in range(4):
            nc.tensor.matmul(pt4[:, m, :], jb1[:, m * 128:(m + 1) * 128],
                             one_r, start=True, stop=False, **kw)
            for k in range(4):
                nc.tensor.matmul(pt4[:, m, :], w1[:, k, m * 128:(m + 1) * 128],
                                 hTc[:, :, k],
                                 start=False, stop=(k == 3), **kw)
        nc.vector.tensor_scalar_max(
            h1[:].rearrange("p m b -> p (m b)"),
            pt4[:].rearrange("p m b -> p (m b)"), 0.0)
        h2 = hpool.tile([128, 2, B_SH], f16)
        pt2 = psum_mlp.tile([128, 4, B_SH], f32, tag="mlp")
        for m in range(2):
            nc.tensor.matmul(pt2[:, m, :], jb2[:, m * 128:(m + 1) * 128],
                             one_r, start=True, stop=False, **kw)
            for k in range(4):
                nc.tensor.matmul(pt2[:, m, :], w2[:, k, m * 128:(m + 1) * 128],
                                 h1[:, k, :], start=False, stop=(k == 3), **kw)
        nc.vector.tensor_scalar_max(
            h2[:].rearrange("p m b -> p (m b)"),
            pt2[:, 0:2, :].rearrange("p m b -> p (m b)"), 0.0)
        h3 = hpool.tile([128, B_SH], f16)
        pt1 = psum_mlp.tile([128, 4, B_SH], f32, tag="mlp")
        nc.tensor.matmul(pt1[:, 0, :], jb3, one_r, start=True, stop=False, **kw)
        for k in range(2):
            nc.tensor.matmul(pt1[:, 0, :], w34[:, k * 128:(k + 1) * 128],
                             h2[:, k, :], start=False, stop=(k == 1), **kw)
        nc.vector.tensor_scalar_max(h3, pt1[:, 0, :], 0.0)
        pt0 = psum_mlp.tile([128, 4, B_SH], f32, tag="mlp")
        nc.tensor.matmul(pt0[0:64, 0, :], jb4, one_r, start=True, stop=False,
                         **kw)
        nc.tensor.matmul(pt0[0:64, 0, :], w34[:, 256:320], h3, start=False,
                         stop=True, **kw)
        nc.vector.tensor_scalar_max(hhg[0:64, :], pt0[0:64, 0, :], 0.0)

        # GRU hidden-path init matmuls (f32 operands, one-time).
        nc.tensor.matmul(prz, whh[:, 0:128], hhg, start=False, stop=False, **kw)
        nc.tensor.matmul(phn, whh[:, 128:192], hhg, start=True, stop=False, **kw)
        nc.tensor.matmul(pd1, ow1, hhg, start=True, stop=False, **kw)
        phn_s = gwork.tile([64, B_SH], f16, tag="phn_s")
        nc.vector.tensor_copy(phn_s, phn)
        pin_s = gwork.tile([64, B_SH], f16, tag="pin_s")
        nc.vector.tensor_copy(pin_s, pin)

        # --- GRU: persistent psum accumulators, 8 unrolled steps.
        # DVE runs the elementwise chain (PSUM-capable); ACT does
        # sigmoid/tanh.  The sigmoid output lands in PSUM (cheaper ACT
        # access), reusing the idle MLP banks.  Each step's x-output add
        # + DMA is deferred into the next step (after its r*h_n) so it
        # never sits ahead of the chain in the DVE queue.
        x_state = [None]
        pending = None

        def emit_x(pd3, t):
            x_new = xpool.tile([3, B_SH], f32, tag="x")
            if x_state[0] is None:
                nc.vector.tensor_copy(x_new, pd3)
            else:
                nc.vector.tensor_add(x_new, x_state[0], pd3)
            nc.sync.dma_start(out=out_d[3 * t:3 * t + 3, :], in_=x_new)
            x_state[0] = x_new

        for t in range(T):
            last = t == T - 1
            r_t = gwork.tile([64, B_SH], f16, tag="r_t")
            nc.scalar.activation(r_t, prz[0:64, :], AF.Sigmoid)
            # zc = 1 - z = sigmoid(-prz_z), off the critical chain
            zc = gwork.tile([64, B_SH], f16, tag="zc")
            nc.scalar.activation(zc, prz[64:128, :], AF.Sigmoid, scale=-1.0)
            tmp = gwork.tile([64, B_SH], f16, tag="tmp")
            nc.vector.tensor_mul(tmp, r_t, phn_s)           # r * h_n
            ptm = gwork.tile([64, B_SH], f16, tag="ptm")
            nc.vector.tensor_add(ptm, tmp, pin_s)           # + i_n
            if pending is not None:
                emit_x(*pending)
                pending = None
            n_t = gwork.tile([64, B_SH], f16, tag="n_t")
            nc.scalar.activation(n_t, ptm, AF.Tanh)
            t1 = gwork.tile([64, B_SH], f16, tag="t1")
            nc.vector.tensor_sub(t1, hhg[0:64, :], n_t)     # hh - n
            dlt = gwork.tile([64, B_SH], f16, tag="dlt")
            nc.vector.tensor_mul(dlt, zc, t1)               # d = (1-z)(hh-n)

            # hh' = hh - d; pd1 first (it gates the output path); the
            # whhn updates are ready before the relu, so PE runs them
            # during the relu's sem latency.
            nc.tensor.matmul(pd1, ow1n, dlt,
                             start=False, stop=last, **kw)
            if not last:
                nc.tensor.matmul(prz, whhn[:, 0:128], dlt,
                                 start=False, stop=False, **kw)
                nc.tensor.matmul(phn, whhn[:, 128:192], dlt,
                                 start=False, stop=(t == T - 2), **kw)
                nc.vector.tensor_sub(hhg[0:64, :], hhg[0:64, :], dlt)
                phn_s = gwork.tile([64, B_SH], f16, tag="phn_s")
                nc.vector.tensor_copy(phn_s, phn)
            nc.vector.tensor_scalar_max(d1g[0:4, :], pd1, 0.0)  # d1(hh')
            if not last:
                # x-recurrence folded through d1g
                nc.tensor.matmul(prz, wixo[:, 0:128], d1g,
                                 start=False, stop=(t == T - 2), **kw)
                nc.tensor.matmul(pin, wixo[:, 128:192], d1g,
                                 start=False, stop=(t == T - 2), **kw)
                pin_s = gwork.tile([64, B_SH], f16, tag="pin_s")
                nc.vector.tensor_copy(pin_s, pin)

            # x output (off the critical chain; flushed next iteration)
            pd3 = psum_gru.tile([3, B_SH], f32, tag="pd3")
            nc.tensor.matmul(pd3, ow23, d1g, start=True, stop=True)
            pending = (pd3, t)
        emit_x(*pending)

    nc.compile()
    return nc


def _get_program():
    if "nc" not in _CACHE:
        _CACHE["nc"] = _build_program()
    return _CACHE["nc"]


def make_in_maps(**inputs) -> list[dict]:
    """Host-side packing + data-parallel sharding -> one in_map per core."""
    f = lambda a: np.ascontiguousarray(np.asarray(a, dtype=np.float32))
    z = f(inputs["z"]).reshape(B, C, S)
    gp = f(inputs["goal_point"])
    gps = f(inputs["goal_point_speed"])
    W_ih, W_hh = f(inputs["W_ih"]), f(inputs["W_hh"])
    b_ih, b_hh = f(inputs["b_ih"]), f(inputs["b_hh"])
    oW1, ob1 = f(inputs["oW1"]), f(inputs["ob1"])
    oW2, ob2 = f(inputs["oW2"]), f(inputs["ob2"])
    oW3, ob3 = f(inputs["oW3"]), f(inputs["ob3"])

    # layer-1 weight: fold the 1/S mean scale and the z-layout channel
    # permutation (chunk j, partition p <-> channel 4p+j).
    jw1t = f(inputs["jW1"]).T * np.float32(1.0 / S)
    perm = (4 * np.arange(128)[None, :] + np.arange(4)[:, None]).reshape(-1)
    w1 = np.ascontiguousarray(jw1t[perm]).astype(np.float16)
    w2 = np.ascontiguousarray(f(inputs["jW2"]).T).astype(np.float16)
    jw3t = f(inputs["jW3"]).T.astype(np.float16)                 # [256, 128]
    jw4t = f(inputs["jW4"]).T.astype(np.float16)                 # [128, 64]
    w34 = np.zeros((128, 320), np.float16)
    w34[:, 0:128] = jw3t[0:128]
    w34[:, 128:256] = jw3t[128:256]
    w34[:, 256:320] = jw4t

    # bias row pack [1, 1024] f16: jb1|jb2|jb3|jb4|ones|pad
    wsm = np.zeros((1, 1024), np.float16)
    wsm[0, 0:512] = f(inputs["jb1"])
    wsm[0, 512:768] = f(inputs["jb2"])
    wsm[0, 768:896] = f(inputs["jb3"])
    wsm[0, 896:960] = f(inputs["jb4"])
    wsm[0, 960:992] = 1.0

    brow = np.concatenate([b_ih[0:128] + b_hh[0:128], b_ih[128:192]])
    wgobt = np.concatenate([W_ih[:, 3:6].T, brow[None, :]])  # [4, 192]
    brow2 = np.concatenate([np.zeros(128, np.float32), b_hh[128:192]])
    whhbt = np.concatenate([W_hh.T, brow2[None, :]])         # [65, 192]
    ow1bt = np.concatenate([oW1.T, ob1[None, :]])            # [65, 4]

    w23 = oW2.T @ oW3.T                                      # [4, 3]
    b23 = ob2 @ oW3.T + ob3                                  # [3]
    wp16 = np.zeros((65, P16_COLS), np.float16)
    wp16[0:64, 0:192] = -W_hh.T
    wp16[0:4, 192:384] = w23 @ W_ih[:, 0:3].T
    wp16[32, 192:384] = W_ih[:, 0:3] @ b23
    wp16[0:64, 384:388] = -oW1.T
    wp16[0:4, 388:391] = w23
    wp16[32, 388:391] = b23
    wp16[0:64, 391:455] = np.eye(64, dtype=np.float16)
    wp16[0:64, 455:519] = -np.eye(64, dtype=np.float16)
    wp16[0:65, 519:711] = whhbt
    wp16[0:4, 711:903] = wgobt
    wp16[0:65, 935:939] = ow1bt

    goalT = np.stack([gp[:, 0, 3], gp[:, 1, 3], gps])        # [3, 256]

    z16 = z.astype(np.float16)

    in_maps = []
    for i in range(N_CORES):
        sl = slice(i * B_SH, (i + 1) * B_SH)
        wpc = wp16.copy()
        wpc[0:3, 903:903 + B_SH] = goalT[:, sl]
        wpc[3, 903:903 + B_SH] = 1.0
        in_maps.append(dict(
            z=np.ascontiguousarray(z16[sl].reshape(ROWS, S)),
            w1=w1, w2=w2, w34=w34, wsm=wsm,
            wp16=wpc,
        ))
    return in_maps


def unshard_out(results: list[dict]) -> np.ndarray:
    # per-core out [24, 32]: row 3t+c, col b  ->  [32, 8, 3]
    parts = [r["out"].reshape(T, 3, B_SH).transpose(2, 0, 1) for r in results]
    return np.ascontiguousarray(np.concatenate(parts, axis=0), dtype=np.float32)


def kernel(**inputs) -> np.ndarray:
    from concourse.bass_utils import run_bass_kernel_spmd

    nc = _get_program()
    in_maps = make_in_maps(**inputs)
    res = run_bass_kernel_spmd(nc, in_maps, core_ids=list(range(N_CORES)))
    return unshard_out(res.results)


# revision 42
# speedup vs baseline: 1.0053x; 1.0053x over previous
"""Trainium2 Bass kernel for nn_ImitationHead (dense_mlp).

Computation (per batch row b of 256):
  h  = mean(z[b], spatial)                # [512] <- z [512,16,16]
  h  = relu-MLP chain 512->512->256->128->64
  goal = [goal_point[b,0,3], goal_point[b,1,3], goal_point_speed[b]]
  GRU (hidden 64, input [x(3); goal(3)]) unrolled 8 steps, each step
  followed by an output MLP 64->4(relu)->4->3 producing dx; x += dx.
  Output: the 8 x values -> [256, 8, 3].

Sharding: pure data parallel, batch 256 -> 8 cores x 32.

Key layout/perf choices:
  - z and all weights travel as float16: halves the HBM traffic that
    dominates the kernel (8 MiB z + ~0.9 MiB weights per core).  fp16
    keeps 10 mantissa bits so the 2e-2 output tolerance is safe
    (measured rel err ~7e-4).
  - on-chip layout fully "transposed" (features on partitions, batch on
    the free axis); z shard viewed as [16384, 256] f16, 7 DMAs of
    [128p, 4, 4, 256] (1 MiB) + 2 half-size DMAs, with 2 KiB contiguous
    DRAM runs.  The channel permutation (chunk j, partition p <->
    channel 4p+j) is undone by permuting W1's rows on the host.
  - spatial sums: TensorReduce runs at 1x on DVE but fp16 TensorTensor
    gets the 2x perf mode, so most chunks reduce via a binary tree of
    fp16 adds on DVE (4 halving stages + one 16-wide reduce); the
    remainder go to ACT as Copy+accum_out chunks.  Both engines stay
    just under the 2.9us-per-DMA stream rate, and the last z DMA is
    split in half so the final tree only trails the stream slightly.
  - weights are loaded AFTER the z stream (they are not needed until
    then), ordered by first use: w1, biases, w2, w34, GRU pack.
  - join MLP in fp16 (1 PE cycle/row): biases fold in as K=1 matmuls
    (bias row x ones row) accumulating into the same psum bank as the
    layer, all m-groups of a layer share ONE psum tile, and a single
    DVE max(.,0) per layer emits the fp16 activations.
  - GRU: persistent PSUM accumulators; hh' = hh - d with
    d = (1-z)*(hh-n); the x-recurrence folds through the output MLP
    (wixo = W23 @ W_ihx.T applied to relu(pd1)).  Per step: ACT does
    sigmoid(r), sigmoid(-prz_z) (= 1-z, off-chain) and tanh; the whole
    elementwise chain runs on DVE in fp16 SBUF tiles (2x perf mode,
    same-engine ordering instead of cross-engine semaphores).  The
    i_n / h_n psum accumulators are shadowed into fp16 SBUF copies
    off-chain right after their matmul updates, so no chain op ever
    waits on a PE semaphore or pays a PSUM access except the forced
    pd1 relu.  Per-step incremental matmuls are fp16.  Biases fold in
    as an extra all-ones input row at init; the 4->4 and 4->3 output
    layers fold into one 4->3 matrix on the host; the spatial-mean
    1/256 folds into W1.
"""

import numpy as np
from contextlib import ExitStack

N_CORES = 8
B = 256
B_SH = B // N_CORES       # 32 batch rows per core
C = 512                   # channels
S = 256                   # spatial 16*16
HID = 64
T = 8                     # pred_len
ROWS = B_SH * C           # 16384 z rows per core
N_DMA = 8                 # z DMAs per core (1 MiB f16 each)
H_PER = 4                 # batch blocks per z DMA
J = 4                     # 256-chunks per partition per batch block

# f16 constant pack [65, 939]
P16_COLS = 939

_CACHE: dict = {}


def _build_program():
    import concourse.bacc as bacc
    import concourse.tile as tile
    from concourse import mybir

    f32 = mybir.dt.float32
    f16 = mybir.dt.float16
    AF = mybir.ActivationFunctionType
    AX = mybir.AxisListType
    ALU = mybir.AluOpType

    nc = bacc.Bacc("TRN2", target_bir_lowering=False, debug=False)

    z = nc.dram_tensor("z", [ROWS, S], f16, kind="ExternalInput")
    w1d = nc.dram_tensor("w1", [512, 512], f16, kind="ExternalInput")
    w2d = nc.dram_tensor("w2", [512, 256], f16, kind="ExternalInput")
    w34d = nc.dram_tensor("w34", [128, 320], f16, kind="ExternalInput")
    wsm_d = nc.dram_tensor("wsm", [1, 1024], f16, kind="ExternalInput")
    wp16_d = nc.dram_tensor("wp16", [65, P16_COLS], f16, kind="ExternalInput")
    out_d = nc.dram_tensor("out", [3 * T, B_SH], f32, kind="ExternalOutput")

    with tile.TileContext(nc) as tc, ExitStack() as ctx, \
            nc.allow_low_precision(reason="fp16 pipeline; output tol 2e-2"):
        consts = ctx.enter_context(tc.tile_pool(name="consts", bufs=1))
        zpool = ctx.enter_context(tc.tile_pool(name="zpool", bufs=3))
        hpool = ctx.enter_context(tc.tile_pool(name="hpool", bufs=1))
        work = ctx.enter_context(tc.tile_pool(name="work", bufs=2))
        gwork = ctx.enter_context(tc.tile_pool(name="gwork", bufs=8))
        xpool = ctx.enter_context(tc.tile_pool(name="xpool", bufs=2))
        psum_mlp = ctx.enter_context(
            tc.tile_pool(name="psum_mlp", bufs=2, space="PSUM"))
        psum_gru = ctx.enter_context(
            tc.tile_pool(name="psum_gru", bufs=1, space="PSUM"))

        wsm = consts.tile([1, 1024], f16)
        wp16 = consts.tile([65, P16_COLS], f16)

        whhn = wp16[0:64, 0:192]
        wixo = wp16[0:33, 192:384]
        ow1n = wp16[0:64, 384:388]
        ow23 = wp16[0:33, 388:391]
        id64 = wp16[0:64, 391:455]
        id64n = wp16[0:64, 455:519]
        whh = wp16[0:65, 519:711]
        wgo = wp16[0:4, 711:903]
        gl = wp16[0:4, 903:903 + B_SH]
        ow1 = wp16[0:65, 935:939]

        # ACT table warmup: sigmoid/tanh tables resident before the tail.
        warm = consts.tile([1, 1], f32)
        nc.vector.memset(warm, 0.0)
        nc.scalar.activation(warm, warm, AF.Sigmoid)
        nc.scalar.activation(warm, warm, AF.Tanh)
        # PE p-state warmup: one early matmul starts the clock-ramp window
        # so the MLP's matmuls run at full speed (prz is reset by its
        # start=True init matmul later).
        warm16 = consts.tile([1, 1], f16)
        nc.vector.memset(warm16, 0.0)

        # hhg rows 0:64 = GRU hidden state (in-place across steps), row 64 = 1.
        hhg = hpool.tile([65, B_SH], f16)
        nc.vector.memset(hhg[64:65, :], 1.0)
        # d1g: relu(pd1) with ones row at partition 32; rows 4:32 stay zero
        # so the K=33 matmuls see only d1 + bias.
        d1g = hpool.tile([33, B_SH], f16)
        nc.vector.memset(d1g[0:33, :], 0.0)
        nc.vector.memset(d1g[32:33, :], 1.0)

        kw = dict(skip_group_check=True)
        prz = psum_gru.tile([128, B_SH], f32, tag="prz")   # r/z pre-act
        nc.tensor.matmul(prz[0:1, 0:1], warm16, warm16, start=True, stop=True,
                         **kw)
        pin = psum_gru.tile([64, B_SH], f32, tag="pin")    # i_n pre-act
        phn = psum_gru.tile([64, B_SH], f32, tag="phn")    # h_n pre-act
        pd1 = psum_gru.tile([4, B_SH], f32, tag="pd1")     # oW1@hh+ob1

        # --- z stream: 8 x 1MiB f16 DMAs.  TensorReduce runs at 1x on
        # DVE but TensorTensor fp16 gets the 2x perf mode, so the spatial
        # sum is mostly a binary tree of fp16 adds on DVE (13 of 16 rows
        # per DMA); the remaining 3 chunks go to ACT (Copy+accum_out).
        # Both engines stay just under the 2.9us DMA time, and the last
        # DMA is split in half so the final tree only trails the stream
        # slightly.
        # Row d*2048 + h*512 + 4p + j -> batch b = 4d+h, channel 4p+j.
        hTc = hpool.tile([128, B_SH, J], f16)
        junk_a = hpool.tile([128, S], f16)
        z_r = z[:].rearrange("(d h p j) s -> d p h j s", h=H_PER, p=128, j=J)

        def tree_reduce(src_ap, n_rows, out_ap):
            # src_ap [128, n_rows, S] f16 -> out_ap [128, n_rows, 1]:
            # 4 halving TT stages then one 1x multi-axis reduce of 16.
            scA = work.tile([128, n_rows, S // 2], f16, tag=f"trA{n_rows}")
            scB = work.tile([128, n_rows, S // 4], f16, tag=f"trB{n_rows}")
            n = S // 2
            nc.vector.tensor_add(scA[:, :, 0:n], src_ap[:, :, 0:n],
                                 src_ap[:, :, n:2 * n])
            # second PE keep-warm beat per DMA (~1.2us offset) so the
            # clock-ramp window survives the stream at full p-state
            nc.tensor.matmul(prz[0:1, 0:1], scA[0:1, 0, 0:1], warm16,
                             start=True, stop=True, **kw)
            cur, oth = scA, scB
            while n > 16:
                h_n = n // 2
                nc.vector.tensor_add(oth[:, :, 0:h_n], cur[:, :, 0:h_n],
                                     cur[:, :, h_n:n])
                cur, oth = oth, cur
                n = h_n
            nc.vector.tensor_reduce(out=out_ap, in_=cur[:, :, 0:16],
                                    axis=AX.X, op=ALU.add)

        def act_chunks(zt, h, b, js):
            for j in js:
                nc.scalar.activation(
                    out=junk_a, in_=zt[:, h, j, :], func=AF.Copy,
                    accum_out=hTc[:, b, j:j + 1])

        for d in range(N_DMA - 1):
            zt = zpool.tile([128, H_PER, J, S], f16, tag="zt")
            nc.sync.dma_start(out=zt, in_=z_r[d])
            b = H_PER * d
            tree_reduce(
                zt[:].rearrange("p h j s -> p (h j) s")[:, 0:13, :], 13,
                hTc[:, b:b + 4, :].rearrange("p b j -> p (b j) ()")[:, 0:13, :])
            act_chunks(zt, 3, b + 3, range(1, 4))
            # keep the PE clock-ramp window alive through the stream
            nc.tensor.matmul(prz[0:1, 0:1], zt[0:1, 0, 0, 0:1], warm16,
                             start=True, stop=True, **kw)
        d = N_DMA - 1
        for half in range(2):
            zh = zpool.tile([128, 2, J, S], f16, tag="zh")
            nc.sync.dma_start(out=zh, in_=z_r[d][:, 2 * half:2 * half + 2])
            b = H_PER * d + 2 * half
            # DVE: row 0 fully + half of row 1; ACT: the other half
            tree_reduce(
                zh[:].rearrange("p h j s -> p (h j) s")[:, 0:6, :], 6,
                hTc[:, b:b + 2, :].rearrange("p b j -> p (b j) ()")[:, 0:6, :])
            act_chunks(zh, 1, b + 1, range(2, 4))
        # --- weights queued after the z stream, in order of first use:
        # w1, biases (L1 relu), w2, w34, then the GRU packs.
        w1 = consts.tile([128, 4, 512], f16)
        jw1_r = w1d[:].rearrange("(k p) m -> k p m", p=128)
        for k in range(0, 4, 2):
            nc.sync.dma_start(out=w1[:, k:k + 2, :], in_=jw1_r[k:k + 2])
        nc.sync.dma_start(out=wsm, in_=wsm_d[:])
        w2 = consts.tile([128, 4, 256], f16)
        nc.sync.dma_start(out=w2, in_=w2d[:].rearrange("(k p) m -> p k m", p=128))
        w34 = consts.tile([128, 320], f16)
        nc.sync.dma_start(out=w34, in_=w34d[:])
        nc.sync.dma_start(out=wp16, in_=wp16_d[:])
        # GRU goal-path init matmuls
        nc.tensor.matmul(prz, wgo[:, 0:128], gl, start=True, stop=False, **kw)
        nc.tensor.matmul(pin, wgo[:, 128:192], gl, start=True, stop=False, **kw)

        # --- join MLP (transposed): hN_T = relu(W @ h_T + b) ---
        # bias+relu fused on Pool: (psum + bias) max 0 -> f16
        # --- join MLP: bias folded in as K=1 matmuls (bias x ones row),
        # all m-groups of a layer accumulate into ONE psum bank, and a
        # single DVE max(.,0) per layer writes the fp16 activations.
        jb1 = wsm[0:1, 0:512]
        jb2 = wsm[0:1, 512:768]
        jb3 = wsm[0:1, 768:896]
        jb4 = wsm[0:1, 896:960]
        one_r = wsm[0:1, 960:992]

        h1 = hpool.tile([128, 4, B_SH], f16)
        pt4 = psum_mlp.tile([128, 4, B_SH], f32, tag="mlp")
        for m in range(4):
            nc.tensor.matmul(pt4[:, m, :], jb1[:, m * 128:(m + 1) * 128],
                             one_r, start=True, stop=False, **kw)
            for k in range(4):
                nc.tensor.matmul(pt4[:, m, :], w1[:, k, m * 128:(m + 1) * 128],
                                 hTc[:, :, k],
                                 start=False, stop=(k == 3), **kw)
        nc.vector.tensor_scalar_max(
            h1[:].rearrange("p m b -> p (m b)"),
            pt4[:].rearrange("p m b -> p (m b)"), 0.0)
        h2 = hpool.tile([128, 2, B_SH], f16)
        pt2 = psum_mlp.tile([128, 4, B_SH], f32, tag="mlp")
        for m in range(2):
            nc.tensor.matmul(pt2[:, m, :], jb2[:, m * 128:(m + 1) * 128],
                             one_r, start=True, stop=False, **kw)
            for k in range(4):
                nc.tensor.matmul(pt2[:, m, :], w2[:, k, m * 128:(m + 1) * 128],
                                 h1[:, k, :], start=False, stop=(k == 3), **kw)
        nc.vector.tensor_scalar_max(
            h2[:].rearrange("p m b -> p (m b)"),
            pt2[:, 0:2, :].rearrange("p m b -> p (m b)"), 0.0)
        h3 = hpool.tile([128, B_SH], f16)
        pt1 = psum_mlp.tile([128, 4, B_SH], f32, tag="mlp")
        nc.tensor.matmul(pt1[:, 0, :], jb3, one_r, start=True, stop=False, **kw)
        for k in range(2):
            nc.tensor.matmul(pt1[:, 0, :], w34[:, k * 128:(k + 1) * 128],
                             h2[:, k, :], start=False, stop=(k == 1), **kw)
        nc.vector.tensor_scalar_max(h3, pt1[:, 0, :], 0.0)
        pt0 = psum_mlp.tile([128, 4, B_SH], f32, tag="mlp")
        nc.tensor.matmul(pt0[0:64, 0, :], jb4, one_r, start=True, stop=False,
                         **kw)
        nc.tensor.matmul(pt0[0:64, 0, :], w34[:, 256:320], h3, start=False,
                         stop=True, **kw)
        nc.vector.tensor_scalar_max(hhg[0:64, :], pt0[0:64, 0, :], 0.0)

        # GRU hidden-path init matmuls (f32 operands, one-time).
        nc.tensor.matmul(prz, whh[:, 0:128], hhg, start=False, stop=False, **kw)
        nc.tensor.matmul(phn, whh[:, 128:192], hhg, start=True, stop=False, **kw)
        nc.tensor.matmul(pd1, ow1, hhg, start=True, stop=False, **kw)
        phn_s = gwork.tile([64, B_SH], f16, tag="phn_s")
        nc.vector.tensor_copy(phn_s, phn)
        pin_s = gwork.tile([64, B_SH], f16, tag="pin_s")
        nc.vector.tensor_copy(pin_s, pin)

        # --- GRU: persistent psum accumulators, 8 unrolled steps.
        # DVE runs the elementwise chain (PSUM-capable); ACT does
        # sigmoid/tanh.  The sigmoid output lands in PSUM (cheaper ACT
        # access), reusing the idle MLP banks.  Each step's x-output add
        # + DMA is deferred into the next step (after its r*h_n) so it
        # never sits ahead of the chain in the DVE queue.
        x_state = [None]
        pending = None

        def emit_x(pd3, t):
            x_new = xpool.tile([3, B_SH], f32, tag="x")
            if x_state[0] is None:
                nc.vector.tensor_copy(x_new, pd3)
            else:
                nc.vector.tensor_add(x_new, x_state[0], pd3)
            nc.sync.dma_start(out=out_d[3 * t:3 * t + 3, :], in_=x_new)
            x_state[0] = x_new

        for t in range(T):
            last = t == T - 1
            r_t = gwork.tile([64, B_SH], f16, tag="r_t")
            nc.scalar.activation(r_t, prz[0:64, :], AF.Sigmoid)
            # zc = 1 - z = sigmoid(-prz_z), off the critical chain
            zc = gwork.tile([64, B_SH], f16, tag="zc")
            nc.scalar.activation(zc, prz[64:128, :], AF.Sigmoid, scale=-1.0)
            tmp = gwork.tile([64, B_SH], f16, tag="tmp")
            nc.vector.tensor_mul(tmp, r_t, phn_s)           # r * h_n
            ptm = gwork.tile([64, B_SH], f16, tag="ptm")
            nc.vector.tensor_add(ptm, tmp, pin_s)           # + i_n
            if pending is not None:
                emit_x(*pending)
                pending = None
            n_t = gwork.tile([64, B_SH], f16, tag="n_t")
            nc.scalar.activation(n_t, ptm, AF.Tanh)
            t1 = gwork.tile([64, B_SH], f16, tag="t1")
            nc.vector.tensor_sub(t1, hhg[0:64, :], n_t)     # hh - n
            dlt = gwork.tile([64, B_SH], f16, tag="dlt")
            nc.vector.tensor_mul(dlt, zc, t1)               # d = (1-z)(hh-n)

            # hh' = hh - d; pd1 first (it gates the output path); the
            # whhn updates are ready before the relu, so PE runs them
            # during the relu's sem latency.
            nc.tensor.matmul(pd1, ow1n, dlt,
                             start=False, stop=last, **kw)
            if not last:
                nc.tensor.matmul(prz, whhn[:, 0:128], dlt,
                                 start=False, stop=False, **kw)
                nc.tensor.matmul(phn, whhn[:, 128:192], dlt,
                                 start=False, stop=(t == T - 2), **kw)
                nc.vector.tensor_sub(hhg[0:64, :], hhg[0:64, :], dlt)
                phn_s = gwork.tile([64, B_SH], f16, tag="phn_s")
                nc.vector.tensor_copy(phn_s, phn)
            nc.vector.tensor_scalar_max(d1g[0:4, :], pd1, 0.0)  # d1(hh')
            if not last:
                # x-recurrence folded through d1g
                nc.tensor.matmul(prz, wixo[:, 0:128], d1g,
                                 start=False, stop=(t == T - 2), **kw)
                nc.tensor.matmul(pin, wixo[:, 128:192], d1g,
                                 start=False, stop=(t == T - 2), **kw)
                pin_s = gwork.tile([64, B_SH], f16, tag="pin_s")
                nc.vector.tensor_copy(pin_s, pin)

            # x output (off the critical chain; flushed next iteration)
            pd3 = psum_gru.tile([3, B_SH], f32, tag="pd3")
            nc.tensor.matmul(pd3, ow23, d1g, start=True, stop=True)
            pending = (pd3, t)
        emit_x(*pending)

    nc.compile()
    return nc


def _get_program():
    if "nc" not in _CACHE:
        _CACHE["nc"] = _build_program()
    return _CACHE["nc"]


def make_in_maps(**inputs) -> list[dict]:
    """Host-side packing + data-parallel sharding -> one in_map per core."""
    f = lambda a: np.ascontiguousarray(np.asarray(a, dtype=np.float32))
    z = f(inputs["z"]).reshape(B, C, S)
    gp = f(inputs["goal_point"])
    gps = f(inputs["goal_point_speed"])
    W_ih, W_hh = f(inputs["W_ih"]), f(inputs["W_hh"])
    b_ih, b_hh = f(inputs["b_ih"]), f(inputs["b_hh"])
    oW1, ob1 = f(inputs["oW1"]), f(inputs["ob1"])
    oW2, ob2 = f(inputs["oW2"]), f(inputs["ob2"])
    oW3, ob3 = f(inputs["oW3"]), f(inputs["ob3"])

    # layer-1 weight: fold the 1/S mean scale and the z-layout channel
    # permutation (chunk j, partition p <-> channel 4p+j).
    jw1t = f(inputs["jW1"]).T * np.float32(1.0 / S)
    perm = (4 * np.arange(128)[None, :] + np.arange(4)[:, None]).reshape(-1)
    w1 = np.ascontiguousarray(jw1t[perm]).astype(np.float16)
    w2 = np.ascontiguousarray(f(inputs["jW2"]).T).astype(np.float16)
    jw3t = f(inputs["jW3"]).T.astype(np.float16)                 # [256, 128]
    jw4t = f(inputs["jW4"]).T.astype(np.float16)                 # [128, 64]
    w34 = np.zeros((128, 320), np.float16)
    w34[:, 0:128] = jw3t[0:128]
    w34[:, 128:256] = jw3t[128:256]
    w34[:, 256:320] = jw4t

    # bias row pack [1, 1024] f16: jb1|jb2|jb3|jb4|ones|pad
    wsm = np.zeros((1, 1024), np.float16)
    wsm[0, 0:512] = f(inputs["jb1"])
    wsm[0, 512:768] = f(inputs["jb2"])
    wsm[0, 768:896] = f(inputs["jb3"])
    wsm[0, 896:960] = f(inputs["jb4"])
    wsm[0, 960:992] = 1.0

    brow = np.concatenate([b_ih[0:128] + b_hh[0:128], b_ih[128:192]])
    wgobt = np.concatenate([W_ih[:, 3:6].T, brow[None, :]])  # [4, 192]
    brow2 = np.concatenate([np.zeros(128, np.float32), b_hh[128:192]])
    whhbt = np.concatenate([W_hh.T, brow2[None, :]])         # [65, 192]
    ow1bt = np.concatenate([oW1.T, ob1[None, :]])            # [65, 4]

    w23 = oW2.T @ oW3.T                                      # [4, 3]
    b23 = ob2 @ oW3.T + ob3                                  # [3]
    wp16 = np.zeros((65, P16_COLS), np.float16)
    wp16[0:64, 0:192] = -W_hh.T
    wp16[0:4, 192:384] = w23 @ W_ih[:, 0:3].T
    wp16[32, 192:384] = W_ih[:, 0:3] @ b23
    wp16[0:64, 384:388] = -oW1.T
    wp16[0:4, 388:391] = w23
    wp16[32, 388:391] = b23
    wp16[0:64, 391:455] = np.eye(64, dtype=np.float16)
    wp16[0:64, 455:519] = -np.eye(64, dtype=np.float16)
    wp16[0:65, 519:711] = whhbt
    wp16[0:4, 711:903] = wgobt
    wp16[0:65, 935:939] = ow1bt

    goalT = np.stack([gp[:, 0, 3], gp[:, 1, 3], gps])        # [3, 256]

    z16 = z.astype(np.float16)

    in_maps = []
    for i in range(N_CORES):
        sl = slice(i * B_SH, (i + 1) * B_SH)
        wpc = wp16.copy()
        wpc[0:3, 903:903 + B_SH] = goalT[:, sl]
        wpc[3, 903:903 + B_SH] = 1.0
        in_maps.append(dict(
            z=np.ascontiguousarray(z16[sl].reshape(ROWS, S)),
            w1=w1, w2=w2, w34=w34, wsm=wsm,
            wp16=wpc,
        ))
    return in_maps


def unshard_out(results: list[dict]) -> np.ndarray:
    # per-core out [24, 32]: row 3t+c, col b  ->  [32, 8, 3]
    parts = [r["out"].reshape(T, 3, B_SH).transpose(2, 0, 1) for r in results]
    return np.ascontiguousarray(np.concatenate(parts, axis=0), dtype=np.float32)


def kernel(**inputs) -> np.ndarray:
    from concourse.bass_utils import run_bass_kernel_spmd

    nc = _get_program()
    in_maps = make_in_maps(**inputs)
    res = run_bass_kernel_spmd(nc, in_maps, core_ids=list(range(N_CORES)))
    return unshard_out(res.results)
